# revision 1
# baseline (speedup 1.0000x reference)
"""GPT2 attention (B=2,S=2048,D=1024,H=16,hd=64, no causal mask) on 8 trn2 cores.

Sharding: core c handles batch b=c//4 and head-group g=c%4 (4 heads).
w_attn columns are split per head group (Q scaled by 1/sqrt(hd) on host);
w_proj rows split per head group; host sums the 4 partial c_proj outputs
per batch (the "all-reduce").

Per-core dataflow (matmuls in float32r, 1 cyc/row at N>=512; every tile a
matmul consumes is written as float32r by its producer so walrus' rounding
check passes):
  hid [2048,1024] --PE transpose--> hidT [1024,2048]
  qkvT[768,2048] = w_slice.T @ hidT   (feature-major Q^T,K^T,V^T, 2 heads/tile)
  V^T --PE transpose--> vaug [k,65] tiles (col 64 = ones for denominator)
  per (head, 512-wide q chunk):
    S^T[k,q] tiles = K^T_tile.T @ Q^T  -> DVE copy to SBUF block [128, 4096]
    one ACT exp per block (amortizes ACT fixed cost; no max-subtraction:
    scores are O(1) so exp is numerically safe)
    O_u^T[65,512] = sum_k vaug.T @ E   (row 64 = softmax denominator)
    obar_h = O_u^T[0:64] * broadcast(1/denom)  (ones-matmul broadcast + DVE mul)
  out[q,1024] = sum_h obar_h.T @ wp_h  (K=64 accumulation, 4 heads)
"""

import sys

import numpy as np

if "/opt/trn_rl_repo" not in sys.path:
    sys.path.insert(0, "/opt/trn_rl_repo")

S = 2048
D = 1024
P = 128
NH = 4  # heads per core
HD = 64
N_CORES = 8

_CACHE = {}


def _build_program():
    import concourse.mybir as mybir
    from concourse import bacc
    from concourse.masks import make_identity
    from concourse.tile import TileContext

    f32r = mybir.dt.float32r
    f32 = mybir.dt.float32
    AF = mybir.ActivationFunctionType
    ALU = mybir.AluOpType

    nc = bacc.Bacc(None, target_bir_lowering=False, debug=False)
    hid = nc.declare_dram_parameter("hid", [S, D], f32r, isOutput=False)
    wqkv = nc.declare_dram_parameter("wqkv", [D, 3 * NH * HD], f32r, isOutput=False)
    wp = nc.declare_dram_parameter("wp", [NH * HD, D], f32r, isOutput=False)
    out = nc.declare_dram_parameter("out", [S, D], f32, isOutput=True)

    with TileContext(nc) as tc:
        with tc.tile_pool(name="const", bufs=1) as constp:
            ident_f = constp.tile([P, P], f32)
            make_identity(nc, ident_f)
            ident = constp.tile([P, P], f32r)
            nc.vector.tensor_copy(ident[:], ident_f[:])
            ones_f = constp.tile([P, HD], f32)
            nc.gpsimd.memset(ones_f[:], 1.0)
            ones_t = constp.tile([P, HD], f32r)
            nc.vector.tensor_copy(ones_t[:], ones_f[:])

            qkvT = [constp.tile([P, S], f32r, name=f"qkvT{i}") for i in range(6)]
            vaug = constp.tile([P, NH * 16 * 65], f32r)

            # ---------------- Stage A: hidT + QKV ----------------
            with tc.tile_pool(name="hidT_pool", bufs=1) as hidTp, \
                 tc.tile_pool(name="stageA", bufs=3) as sA, \
                 tc.tile_pool(name="w_pool", bufs=1) as wpool, \
                 tc.tile_pool(name="tpsum", bufs=3, space="PSUM") as tpsum, \
                 tc.tile_pool(name="qpsum", bufs=3, space="PSUM") as qpsum:
                hidT = [hidTp.tile([P, S], f32r, name=f"hidT{i}") for i in range(8)]
                w_sb = [wpool.tile([P, 768], f32r, name=f"w{i}") for i in range(8)]
                for i in range(8):
                    nc.sync.dma_start(out=w_sb[i][:], in_=wqkv[i * P : (i + 1) * P, :])
                for st in range(16):
                    ht = sA.tile([P, D], f32r, tag="hidload")
                    nc.sync.dma_start(out=ht[:], in_=hid[st * P : (st + 1) * P, :])
                    for dt_ in range(8):
                        tp = tpsum.tile([P, P], f32r, tag="tp")
                        nc.tensor.transpose(
                            tp[:], ht[:, dt_ * P : (dt_ + 1) * P], ident[:]
                        )
                        nc.vector.tensor_copy(
                            hidT[dt_][:, st * P : (st + 1) * P], tp[:]
                        )
                for ct in range(6):
                    for qc in range(4):
                        ps = qpsum.tile([P, 512], f32, tag="qkvps")
                        for dt_ in range(8):
                            nc.tensor.matmul(
                                ps[:],
                                lhsT=w_sb[dt_][:, ct * P : (ct + 1) * P],
                                rhs=hidT[dt_][:, qc * 512 : (qc + 1) * 512],
                                start=(dt_ == 0),
                                stop=(dt_ == 7),
                            )
                        nc.vector.tensor_copy(
                            qkvT[ct][:, qc * 512 : (qc + 1) * 512], ps[:]
                        )
                # V seq-major (transpose V^T) into vaug; col 64 of each 65 = ones
                for h in range(NH):
                    par = HD * (h % 2)
                    vsrc = qkvT[4 + h // 2]
                    for kt in range(16):
                        vp = tpsum.tile([P, P], f32r, tag="tp")
                        nc.tensor.transpose(
                            vp[:, :HD],
                            vsrc[par : par + HD, kt * P : (kt + 1) * P],
                            ident[par : par + HD, par : par + HD],
                        )
                        base = (h * 16 + kt) * 65
                        nc.vector.tensor_copy(vaug[:, base : base + HD], vp[:, :HD])
                        nc.vector.tensor_copy(
                            vaug[:, base + HD : base + 65], ones_f[:, 0:1]
                        )

            # ---------------- Stages B+C ----------------
            with tc.tile_pool(name="persistBC", bufs=1) as perBC:
                obar = [perBC.tile([HD, S], f32r, name=f"obar{i}") for i in range(NH)]
                wp_sb = [perBC.tile([HD, D], f32r, name=f"wp{i}") for i in range(NH)]
                for h in range(NH):
                    nc.sync.dma_start(
                        out=wp_sb[h][:], in_=wp[h * HD : (h + 1) * HD, :]
                    )

                with tc.tile_pool(name="sblk", bufs=3) as sblk, \
                     tc.tile_pool(name="npool", bufs=3) as npool, \
                     tc.tile_pool(name="spsum", bufs=2, space="PSUM") as spsum, \
                     tc.tile_pool(name="opsum", bufs=1, space="PSUM") as opsum, \
                     tc.tile_pool(name="rpsum", bufs=1, space="PSUM") as rpsum:
                    for h in range(NH):
                        par = HD * (h % 2)
                        qT = qkvT[0 + h // 2]
                        kT = qkvT[2 + h // 2]
                        for qc in range(2):
                            q0 = qc * 1024
                            op = opsum.tile([65, 1024], f32, tag="op")
                            for kt in range(16):
                                sp = spsum.tile([P, 1024], f32, tag="sp")
                                for u in range(2):
                                    nc.tensor.matmul(
                                        sp[:, u * 512 : (u + 1) * 512],
                                        lhsT=kT[par : par + HD, kt * P : (kt + 1) * P],
                                        rhs=qT[par : par + HD, q0 + u * 512 : q0 + (u + 1) * 512],
                                        start=True,
                                        stop=True,
                                    )
                                eb = sblk.tile([P, 1024], f32r, tag="sb")
                                nc.scalar.activation(eb[:], sp[:], AF.Exp)
                                base = (h * 16 + kt) * 65
                                for u in range(2):
                                    nc.tensor.matmul(
                                        op[:, u * 512 : (u + 1) * 512],
                                        lhsT=vaug[:, base : base + 65],
                                        rhs=eb[:, u * 512 : (u + 1) * 512],
                                        start=(kt == 0),
                                        stop=(kt == 15),
                                    )
                            rec = npool.tile([P, 1024], f32r, tag="rec")
                            with nc.allow_low_precision(
                                reason="f32r recip of softmax denom"
                            ):
                                nc.vector.reciprocal(rec[64:65, :], op[64:65, :])
                            rb = rpsum.tile([HD, 1024], f32, tag="rb")
                            for u in range(2):
                                nc.tensor.matmul(
                                    rb[:, u * 512 : (u + 1) * 512],
                                    lhsT=ones_t[64:65, :],
                                    rhs=rec[64:65, u * 512 : (u + 1) * 512],
                                    start=True, stop=True,
                                )
                            ou_sb = npool.tile([HD, 1024], f32r, tag="ou")
                            nc.vector.tensor_copy(ou_sb[:], op[0:HD, :])
                            rb_sb = npool.tile([HD, 1024], f32r, tag="rbs")
                            nc.vector.tensor_copy(rb_sb[:], rb[:])
                            with nc.allow_low_precision(
                                reason="softmax normalize in f32r"
                            ):
                                nc.vector.tensor_tensor(
                                    out=obar[h][:, q0 : q0 + 1024],
                                    in0=ou_sb[:],
                                    in1=rb_sb[:],
                                    op=ALU.mult,
                                )

                # ---------------- Stage C: projection ----------------
                with tc.tile_pool(name="outp", bufs=4) as outp, \
                     tc.tile_pool(name="ppsum", bufs=4, space="PSUM") as ppsum:
                    for qt in range(16):
                        ot = outp.tile([P, D], f32, tag="ot")
                        for ec in range(2):
                            pp = ppsum.tile([P, 512], f32, tag="pp")
                            for h in range(NH):
                                nc.tensor.matmul(
                                    pp[:],
                                    lhsT=obar[h][:, qt * P : (qt + 1) * P],
                                    rhs=wp_sb[h][:, ec * 512 : (ec + 1) * 512],
                                    start=(h == 0),
                                    stop=(h == NH - 1),
                                )
                            nc.vector.tensor_copy(
                                ot[:, ec * 512 : (ec + 1) * 512], pp[:]
                            )
                        nc.sync.dma_start(
                            out=out[qt * P : (qt + 1) * P, :], in_=ot[:]
                        )

    nc.compile()
    return nc


def _get_nc():
    if "nc" not in _CACHE:
        _CACHE["nc"] = _build_program()
    return _CACHE["nc"]


def _shard_inputs(hidden_states, w_attn, w_proj):
    scale = 1.0 / np.sqrt(np.float32(HD))
    in_maps = []
    for c in range(N_CORES):
        b, g = divmod(c, 4)
        cs = slice(g * NH * HD, (g + 1) * NH * HD)
        wq = w_attn[:, 0:D][:, cs] * scale
        wk = w_attn[:, D : 2 * D][:, cs]
        wv = w_attn[:, 2 * D : 3 * D][:, cs]
        in_maps.append(
            {
                "hid": np.ascontiguousarray(hidden_states[b], dtype=np.float32),
                "wqkv": np.ascontiguousarray(
                    np.concatenate([wq, wk, wv], axis=1), dtype=np.float32
                ),
                "wp": np.ascontiguousarray(w_proj[cs, :], dtype=np.float32),
            }
        )
    return in_maps


def run(hidden_states, w_attn, w_proj, trace=False):
    from concourse.bass_utils import run_bass_kernel_spmd

    nc = _get_nc()
    in_maps = _shard_inputs(hidden_states, w_attn, w_proj)
    res = run_bass_kernel_spmd(nc, in_maps, list(range(N_CORES)), trace=trace)
    parts = [res.results[c]["out"] for c in range(N_CORES)]
    out = np.stack(
        [
            parts[0] + parts[1] + parts[2] + parts[3],
            parts[4] + parts[5] + parts[6] + parts[7],
        ]
    ).astype(np.float32)
    return out, res


def kernel(hidden_states, w_attn, w_proj):
    out, _ = run(
        np.asarray(hidden_states), np.asarray(w_attn), np.asarray(w_proj)
    )
    return out



# revision 8
# speedup vs baseline: 1.7292x; 1.7292x over previous
"""GPT2 attention (B=2,S=2048,D=1024,H=16,hd=64, no causal mask) on 8 trn2 cores.

Sharding: core c handles batch b=c//4 and head-group g=c%4 (4 heads = 2 pairs).
w_attn columns split per head group (Q scaled by 1/sqrt(hd) on host); w_proj
rows split per head group; host sums the 4 partial c_proj outputs per batch.

v2 dataflow (all bf16 on SBUF, fp32 PSUM accumulation):
  host passes hidT [D,S] (pre-transposed) so no on-device hid transpose.
  qT/kT [128,S] per head-pair = wqk_pair.T @ hidT   (one MM per dt chunk)
  V computed directly seq-major: V[k,f] = hidT_chunk.T @ wv  -> vaug
  scores: per (pair,qc512,kt): two row-tiled concurrent MMs (K=64 each,
    heads at array rows 0-63 / 64-127) -> sp[128,1024] -> one ACT exp
    -> eb bf16 -> attnV MMs accumulate op[65,512] (row 64 = denominator).
  normalize: DVE reciprocal_approx_fast on denom row, ones-matmul broadcast,
    DVE mult -> obar (feature-major, pair-packed 128 rows).
  c_proj per 128-query tile: K=128 chains over 2 pairs, interleaved with
    stage B to keep the PE warm; bf16 partial outputs, host sums in f32.
"""

import sys

import numpy as np

if "/opt/trn_rl_repo" not in sys.path:
    sys.path.insert(0, "/opt/trn_rl_repo")

S = 2048
D = 1024
P = 128
NH = 4  # heads per core
HD = 64
N_CORES = 8

_CACHE = {}


def _build_program():
    import concourse.mybir as mybir
    from concourse import bacc
    from concourse.tile import TileContext

    bf16 = mybir.dt.bfloat16
    f32 = mybir.dt.float32
    AF = mybir.ActivationFunctionType
    ALU = mybir.AluOpType

    nc = bacc.Bacc(None, target_bir_lowering=False, debug=False)
    hidT = nc.declare_dram_parameter("hidT", [D, S], bf16, isOutput=False)
    wqkv = nc.declare_dram_parameter("wqkv", [D, 3 * NH * HD], bf16, isOutput=False)
    wp = nc.declare_dram_parameter("wp", [NH * HD, D], bf16, isOutput=False)
    out = nc.declare_dram_parameter("out", [S, D], bf16, isOutput=True)

    with TileContext(nc) as tc:
        with tc.tile_pool(name="const", bufs=1) as constp:
            vaug = constp.tile([P, NH * 16 * 65], bf16)
            # ones column (col 64 of each 65-block) for the softmax denom
            vaug_on = vaug[:, :].rearrange("p (n c) -> p n c", c=65)[:, :, 64:65]
            nc.gpsimd.memset(vaug_on, 1.0)

            hid_sb = [constp.tile([P, S], bf16, name=f"hid{i}") for i in range(8)]
            w_sb = [constp.tile([P, 768], bf16, name=f"w{i}") for i in range(8)]
            wp_sb = [constp.tile([P, D], bf16, name=f"wp{i}") for i in range(2)]
            qT = [constp.tile([P, S], bf16, name=f"qT{i}") for i in range(2)]
            kT = [constp.tile([P, S], bf16, name=f"kT{i}") for i in range(2)]
            obar = [constp.tile([P, S], bf16, name=f"ob{i}") for i in range(2)]

            for i in range(8):
                nc.sync.dma_start(out=hid_sb[i][:], in_=hidT[i * P : (i + 1) * P, :])
                nc.sync.dma_start(out=w_sb[i][:], in_=wqkv[i * P : (i + 1) * P, :])
            for p in range(2):
                nc.sync.dma_start(
                    out=wp_sb[p][:], in_=wp[p * P : (p + 1) * P, :]
                )

            # ---------------- Stage A: qT/kT per pair + V seq-major ------
            with tc.tile_pool(name="qkpsum", bufs=4, space="PSUM") as qkp, \
                 tc.tile_pool(name="vpsum", bufs=3, space="PSUM") as vp:
                for p in range(2):
                    for col, dst in ((p * P, qT[p]), (256 + p * P, kT[p])):
                        for q4 in range(4):
                            ps = qkp.tile([P, 512], f32, tag="qk")
                            for dt_ in range(8):
                                nc.tensor.matmul(
                                    ps[:],
                                    lhsT=w_sb[dt_][:, col : col + P],
                                    rhs=hid_sb[dt_][:, q4 * 512 : (q4 + 1) * 512],
                                    start=(dt_ == 0),
                                    stop=(dt_ == 7),
                                )
                            with nc.allow_low_precision(reason="bf16 qkT"):
                                nc.vector.tensor_copy(
                                    dst[:, q4 * 512 : (q4 + 1) * 512], ps[:]
                                )
                vaug4 = vaug[:, :].rearrange("p (h x) -> p h x", h=NH)
                for kt in range(16):
                    vps = vp.tile([P, NH * HD], f32, tag="v")
                    for dt_ in range(8):
                        nc.tensor.matmul(
                            vps[:],
                            lhsT=hid_sb[dt_][:, kt * P : (kt + 1) * P],
                            rhs=w_sb[dt_][:, 512:768],
                            start=(dt_ == 0),
                            stop=(dt_ == 7),
                        )
                    src = vps[:, :].rearrange("p (h c) -> p h c", h=NH)
                    dst = vaug4[:, :, kt * 65 : kt * 65 + HD]
                    with nc.allow_low_precision(reason="bf16 V"):
                        nc.vector.tensor_copy(dst, src)

            # ---------------- Stages B + C (interleaved) ----------------
            with tc.tile_pool(name="spsum", bufs=2, space="PSUM") as spsum, \
                 tc.tile_pool(name="opsum", bufs=2, space="PSUM") as opsum, \
                 tc.tile_pool(name="ppsum", bufs=2, space="PSUM") as ppp, \
                 tc.tile_pool(name="ebpool", bufs=3) as ebp, \
                 tc.tile_pool(name="recpool", bufs=2) as recp, \
                 tc.tile_pool(name="rbsb", bufs=2) as rbsbp, \
                 tc.tile_pool(name="otpool", bufs=2) as otp:

                def stage_c(qc):
                    for j in range(4):
                        qt = qc * 4 + j
                        ot = otp.tile([P, D], bf16, tag="ot")
                        for ec in range(2):
                            pp = ppp.tile([P, 512], f32, tag="pp")
                            for p in range(2):
                                nc.tensor.matmul(
                                    pp[:],
                                    lhsT=obar[p][:, qt * P : (qt + 1) * P],
                                    rhs=wp_sb[p][:, ec * 512 : (ec + 1) * 512],
                                    start=(p == 0),
                                    stop=(p == 1),
                                )
                            with nc.allow_low_precision(reason="bf16 out"):
                                nc.vector.tensor_copy(
                                    ot[:, ec * 512 : (ec + 1) * 512], pp[:]
                                )
                        nc.sync.dma_start(
                            out=out[qt * P : (qt + 1) * P, :], in_=ot[:]
                        )

                for qc in range(4):
                    q0 = qc * 512
                    for p in range(2):
                        ops = [
                            opsum.tile([65, 512], f32, tag="op", name=f"op{u}")
                            for u in range(2)
                        ]
                        for kt in range(16):
                            sp = spsum.tile([P, 1024], f32, tag="sp")
                            for u in range(2):
                                r0 = u * HD
                                nc.tensor.matmul(
                                    sp[:, u * 512 : (u + 1) * 512],
                                    lhsT=kT[p][r0 : r0 + HD, kt * P : (kt + 1) * P],
                                    rhs=qT[p][r0 : r0 + HD, q0 : q0 + 512],
                                    start=True,
                                    stop=True,
                                )
                            eb = ebp.tile([P, 1024], bf16, tag="eb")
                            with nc.allow_low_precision(reason="bf16 exp"):
                                nc.scalar.activation(eb[:], sp[:], AF.Exp)
                            for u in range(2):
                                base = ((2 * p + u) * 16 + kt) * 65
                                nc.tensor.matmul(
                                    ops[u][:],
                                    lhsT=vaug[:, base : base + 65],
                                    rhs=eb[:, u * 512 : (u + 1) * 512],
                                    start=(kt == 0),
                                    stop=(kt == 15),
                                )
                        for u in range(2):
                            rec = recp.tile([1, 512], f32, tag="rec")
                            with nc.allow_low_precision(
                                reason="softmax denom recip"
                            ):
                                nc.vector.reciprocal(
                                    rec[:], ops[u][HD : HD + 1, :]
                                )
                            rb_s = rbsbp.tile([HD, 512], f32, tag="rbsb")
                            nc.gpsimd.partition_broadcast(
                                rb_s[:], rec[0:1, :], channels=HD
                            )
                            with nc.allow_low_precision(reason="bf16 obar"):
                                nc.vector.tensor_tensor(
                                    out=obar[p][
                                        u * HD : (u + 1) * HD, q0 : q0 + 512
                                    ],
                                    in0=ops[u][0:HD, :],
                                    in1=rb_s[:],
                                    op=ALU.mult,
                                )
                    if qc > 0:
                        stage_c(qc - 1)
                stage_c(3)

    nc.compile()
    return nc


def _get_nc():
    if "nc" not in _CACHE:
        _CACHE["nc"] = _build_program()
    return _CACHE["nc"]


def _shard_inputs(hidden_states, w_attn, w_proj):
    import ml_dtypes

    bf16 = ml_dtypes.bfloat16
    scale = 1.0 / np.sqrt(np.float32(HD))
    in_maps = []
    for c in range(N_CORES):
        b, g = divmod(c, 4)
        cs = slice(g * NH * HD, (g + 1) * NH * HD)
        wq = w_attn[:, 0:D][:, cs] * scale
        wk = w_attn[:, D : 2 * D][:, cs]
        wv = w_attn[:, 2 * D : 3 * D][:, cs]
        in_maps.append(
            {
                "hidT": np.ascontiguousarray(
                    hidden_states[b].T.astype(bf16)
                ),
                "wqkv": np.ascontiguousarray(
                    np.concatenate([wq, wk, wv], axis=1).astype(bf16)
                ),
                "wp": np.ascontiguousarray(w_proj[cs, :].astype(bf16)),
            }
        )
    return in_maps


def run(hidden_states, w_attn, w_proj, trace=False):
    from concourse.bass_utils import run_bass_kernel_spmd

    nc = _get_nc()
    in_maps = _shard_inputs(hidden_states, w_attn, w_proj)
    res = run_bass_kernel_spmd(nc, in_maps, list(range(N_CORES)), trace=trace)
    parts = [res.results[c]["out"].astype(np.float32) for c in range(N_CORES)]
    out = np.stack(
        [
            parts[0] + parts[1] + parts[2] + parts[3],
            parts[4] + parts[5] + parts[6] + parts[7],
        ]
    ).astype(np.float32)
    return out, res


def kernel(hidden_states, w_attn, w_proj):
    out, _ = run(
        np.asarray(hidden_states), np.asarray(w_attn), np.asarray(w_proj)
    )
    return out
